# revision 1
# baseline (speedup 1.0000x reference)
"""CIGLoss (segment_reduce) Trainium2 kernel.

Strategy (data-parallel over batch, per the sharding hint):
  - Each of the 8 NeuronCores owns one image and that image's pixel list
    (segments are image-local: seg // 500 == image).
  - Host-side sharding packs each image's ~500 segments into a
    [128 partitions, NSLOT slots, L] padded grid (one whole segment per
    slot).  Pad entries point at a zero element appended to the image, so
    they contribute 0 to every sum.
  - The value lookup input[b,0,row,col] happens during host packing (this
    toolchain's walrus mis-lowers per-element indirect DMA: one descriptor
    per contiguous dest run, only the run-start offset honored — verified
    by hardware probes; see hw_gather_probe*.py).  All reductions run on
    device, per-slot:
        sums  = reduce_add(v)            counts = reduce_add(row < H)
        mean  = sums * recip(max(counts,1))
        dev   = reduce_add(|v - mean|)   contrib = dev * recip
    and a final partition reduce to one scalar per core.
  - Host sums the 8 per-core partials and divides by B.
"""

import numpy as np

_NUM_PATHS = 4000
_P = 128  # SBUF partitions


def _build_nc(nslot: int, L: int, ntot: int, W: int, H: int, chunk: int):
    import concourse.bacc as bacc
    import concourse.bass as bass
    import concourse.tile as tile
    from concourse import mybir

    f32 = mybir.dt.float32
    i32 = mybir.dt.int32
    Alu = mybir.AluOpType
    Ax = mybir.AxisListType
    FREE = nslot * L

    assert L % chunk == 0 or chunk % L == 0
    nch = FREE // chunk
    spc = max(1, chunk // L)   # whole slots per chunk (when chunk >= L)
    cps = max(1, L // chunk)   # chunks per slot (when chunk < L)

    u8 = mybir.dt.uint8
    nc = bacc.Bacc("TRN2", debug=False)
    v_d = nc.dram_tensor("vP", [_P, FREE], f32, kind="ExternalInput")
    ind_d = nc.dram_tensor("indP", [_P, FREE], u8, kind="ExternalInput")
    out_d = nc.dram_tensor("out", [_P, 1], f32, kind="ExternalOutput")

    _emit(nc, tile, bass, nslot, L, W, H, chunk, f32, u8, Alu, Ax,
          v_d, ind_d, out_d, FREE, nch, spc, cps)
    # Bacc defers register allocation + wait-splitting to finalize(); the
    # pjrt run path serializes the module as-is, so finalize here.
    nc.finalize()
    return nc


def _emit(nc, tile, bass, nslot, L, W, H, chunk, f32, u8, Alu, Ax,
          v_d, ind_d, out_d, FREE, nch, spc, cps):
    with tile.TileContext(nc) as tc:
        with (
            tc.tile_pool(name="big", bufs=1) as big,
            tc.tile_pool(name="small", bufs=1) as small,
        ):
            # u8 indicator of real (non-pad) pixels, upcast to f32
            ind8_t = big.tile([_P, FREE], u8)
            nc.sync.dma_start(out=ind8_t[:], in_=ind_d[:, :])
            ind_t = big.tile([_P, FREE], f32)
            nc.vector.tensor_copy(out=ind_t[:], in_=ind8_t[:])

            # gathered pixel values in slot layout; chunked load with
            # per-chunk partial sums so load and reduce overlap.
            v_t = big.tile([_P, FREE], f32)
            psum_t = small.tile([_P, nch * spc], f32)
            for k in range(nch):
                a, b = k * chunk, (k + 1) * chunk
                nc.sync.dma_start(out=v_t[:, a:b], in_=v_d[:, a:b])
                nc.vector.tensor_reduce(
                    out=psum_t[:, k * spc:(k + 1) * spc],
                    in_=v_t[:, a:b].rearrange("p (s l) -> p s l", s=spc),
                    axis=Ax.X, op=Alu.add,
                )

            v3 = v_t[:].rearrange("p (s l) -> p s l", s=nslot)
            ind3 = ind_t[:].rearrange("p (s l) -> p s l", s=nslot)

            # combine per-chunk partials into per-slot sums
            sums = small.tile([_P, nslot], f32)
            if cps == 1:
                nc.vector.tensor_copy(out=sums[:], in_=psum_t[:])
            elif cps == 2:
                nc.vector.tensor_tensor(
                    out=sums[:], in0=psum_t[:, 0::2], in1=psum_t[:, 1::2],
                    op=Alu.add,
                )
            else:
                nc.vector.tensor_reduce(
                    out=sums[:],
                    in_=psum_t[:].rearrange("p (s c) -> p s c", s=nslot),
                    axis=Ax.X, op=Alu.add,
                )
            counts = small.tile([_P, nslot], f32)
            nc.vector.tensor_reduce(out=counts[:], in_=ind3, axis=Ax.X, op=Alu.add)
            nc.vector.tensor_scalar_max(counts[:], counts[:], 1.0)
            w_t = small.tile([_P, nslot], f32)
            nc.vector.reciprocal(w_t[:], counts[:])
            means = small.tile([_P, nslot], f32)
            nc.vector.tensor_tensor(
                out=means[:], in0=sums[:], in1=w_t[:], op=Alu.mult
            )

            x_t = big.tile([_P, FREE], f32)
            x3 = x_t[:].rearrange("p (s l) -> p s l", s=nslot)
            nc.vector.tensor_tensor(
                out=x3, in0=v3, in1=means[:].to_broadcast([_P, nslot, L]),
                op=Alu.subtract,
            )
            devs = small.tile([_P, nslot], f32)
            nc.vector.tensor_reduce(
                out=devs[:], in_=x3, axis=Ax.X, op=Alu.add,
                apply_absolute_value=True,
            )
            # pads were gathered as 0, so each contributed |0 - mean| to devs;
            # subtract the known pad contribution (L - count) * |mean|.
            npad = small.tile([_P, nslot], f32)
            nc.vector.tensor_scalar(
                out=npad[:], in0=counts[:], scalar1=-1.0, scalar2=float(L),
                op0=Alu.mult, op1=Alu.add,
            )
            absm = small.tile([_P, nslot], f32)
            nc.vector.tensor_scalar(
                out=absm[:], in0=means[:], scalar1=-1.0, scalar2=None, op0=Alu.mult
            )
            nc.vector.tensor_tensor(
                out=absm[:], in0=absm[:], in1=means[:], op=Alu.max
            )
            nc.vector.tensor_tensor(
                out=npad[:], in0=npad[:], in1=absm[:], op=Alu.mult
            )
            nc.vector.tensor_tensor(
                out=devs[:], in0=devs[:], in1=npad[:], op=Alu.subtract
            )
            contrib = small.tile([_P, nslot], f32)
            nc.vector.tensor_tensor(
                out=contrib[:], in0=devs[:], in1=w_t[:], op=Alu.mult
            )
            part = small.tile([_P, 1], f32)
            nc.vector.tensor_reduce(
                out=part[:], in_=contrib[:], axis=Ax.X, op=Alu.add
            )
            nc.sync.dma_start(out=out_d[:, :], in_=part[:])
    return nc


_CACHE = {}


def _get_nc(key):
    if key not in _CACHE:
        _CACHE[key] = _build_nc(*key)
    return _CACHE[key]


def _pack(input, rows, cols, seg_ids, num_paths):
    """Host-side sharding: one image per core, segments packed into a
    [ncore, 128, nslot*L] padded slot grid."""
    B, C, H, W = input.shape
    ppi = num_paths // B  # paths (segments) per image
    npix = rows.shape[0]

    bnd = np.searchsorted(seg_ids, np.arange(num_paths + 1)).astype(np.int64)
    seg_lens = np.diff(bnd)
    nslot = int(np.ceil(ppi / _P))
    lmax = int(seg_lens.max()) if npix else 1
    L = max(128, int(np.ceil(lmax / 128.0)) * 128)
    FREE = nslot * L

    s = np.arange(num_paths)
    core = s // ppi
    local = s % ppi
    part = local % _P
    slot = local // _P
    base = ((core * _P + part) * np.int64(nslot) + slot) * L
    dest = np.repeat(base, seg_lens) + (
        np.arange(npix, dtype=np.int64) - np.repeat(bnd[:-1], seg_lens)
    )
    ind_p = np.zeros(B * _P * FREE, np.uint8)
    ind_p[dest] = 1
    # Pixel values in slot layout.  This lookup runs on the host: the
    # toolchain's walrus build mis-lowers sub-row indirect DMA (one
    # descriptor per contiguous dest run, only the run-start offset is
    # honored), so a per-element device gather is not expressible; all
    # reductions stay on device.
    core_of = np.repeat(core, seg_lens)
    v_p = np.zeros(B * _P * FREE, np.float32)
    v_p[dest] = input[core_of, 0, rows, cols]
    return (v_p.reshape(B, _P, FREE), ind_p.reshape(B, _P, FREE),
            nslot, L, H * W + 128)


def kernel(input, rows, cols, seg_ids, _trace=False, _num_paths=_NUM_PATHS):
    from concourse.bass_utils import run_bass_kernel_spmd

    input = np.ascontiguousarray(np.asarray(input, np.float32))
    rows = np.ascontiguousarray(np.asarray(rows, np.int32))
    cols = np.ascontiguousarray(np.asarray(cols, np.int32))
    seg_ids = np.ascontiguousarray(np.asarray(seg_ids, np.int32))
    B, C, H, W = input.shape

    v_p, ind_p, nslot, L, ntot = _pack(input, rows, cols, seg_ids, _num_paths)
    chunk = L // 2 if (L % 2 == 0 and L >= 512) else L
    nc = _get_nc((nslot, L, ntot, W, H, chunk))
    in_maps = [
        {"vP": v_p[i], "indP": ind_p[i]} for i in range(B)
    ]
    res = run_bass_kernel_spmd(nc, in_maps, core_ids=list(range(B)), trace=_trace)
    total = sum(float(r["out"].sum()) for r in res.results)
    out = np.float32(total / B)
    if _trace:
        return out, res
    return out



# revision 8
# speedup vs baseline: 1.5122x; 1.5122x over previous
"""CIGLoss (segment_reduce) Trainium2 kernel.

Strategy (data-parallel over batch, per the sharding hint):
  - Each of the 8 NeuronCores owns one image and that image's pixel list
    (segments are image-local: seg // 500 == image).
  - Host-side sharding sorts each image's ~500 segments by length and
    packs them into a [128 partitions, sum(L_s)] fp16 grid: slot s holds
    segments ranked [s*128, (s+1)*128), padded to that slot's own width
    L_s.  Pad entries are 0 and their known contribution |0 - mean| is
    subtracted exactly on device.
  - The value lookup input[b,0,row,col] happens during host packing (this
    toolchain's walrus mis-lowers per-element indirect DMA: one descriptor
    per contiguous dest run, only the run-start offset honored — verified
    by hardware probes).  All heavy reductions run on device:
        sums_s = accum-reduce(v_s)              (DVE fused tensor_scalar)
        mean_s = sums_s * (1/count)             (counts are host metadata)
        dev_s  = accum-reduce(|v_s - mean_s|)   (DVE fused or ACT Abs+accum)
        contrib = (dev_s - npad_s*|mean_s|) / count
    and a final partition reduce to [128,1] per core.
  - Host sums the 8 per-core partials and divides by B.
"""

import numpy as np

_NUM_PATHS = 4000
_P = 128  # SBUF partitions

# dev-pass engine per slot: 'act' (activation Abs + accum) or 'dve'
# (tensor_tensor subtract + tensor_scalar abs accum, two passes).
_DEV_ENG = ("act", "act", "act", "act")
_SUM_MODE = "ts"  # 'ts' (fused tensor_scalar accum) or 'reduce'


def _build_nc(key):
    import concourse.bacc as bacc
    import concourse.bass as bass
    import concourse.tile as tile
    from concourse import mybir

    Ls, dev_eng, sum_mode = key
    f32 = mybir.dt.float32
    f16 = mybir.dt.float16
    Alu = mybir.AluOpType
    Ax = mybir.AxisListType
    Act = mybir.ActivationFunctionType
    ns = len(Ls)
    off = np.concatenate([[0], np.cumsum(Ls)]).astype(int)
    FREE = int(off[-1])
    Lmax = max(Ls)

    nc = bacc.Bacc("TRN2", debug=False)
    v_d = nc.dram_tensor("vP", [_P, FREE], f16, kind="ExternalInput")
    aux_d = nc.dram_tensor("auxP", [_P, 3 * ns], f32, kind="ExternalInput")
    out_d = nc.dram_tensor("out", [_P, 1], f32, kind="ExternalOutput")

    any_act = any(e == "act" for e in dev_eng)

    with tile.TileContext(nc) as tc:
        with (
            tc.tile_pool(name="big", bufs=1) as big,
            tc.tile_pool(name="small", bufs=1) as small,
        ):
            if any_act:
                # trigger the ACT table-set load at t=0 so it overlaps the
                # DMA lead-in instead of stalling the first real dev pass
                warm = small.tile([_P, 1], f32)
                warmacc = small.tile([_P, 1], f32)
                nc.vector.memset(warm[:], 0.0)
                nc.scalar.activation(
                    out=warm[:], in_=warm[:], func=Act.Abs,
                    bias=0.0, scale=1.0, accum_out=warmacc[:],
                )
            aux_t = small.tile([_P, 3 * ns], f32)
            nc.sync.dma_start(out=aux_t[:], in_=aux_d[:, :])
            w_v = aux_t[:, 0:ns]
            negw_v = aux_t[:, ns:2 * ns]
            npad_v = aux_t[:, 2 * ns:3 * ns]

            v_t = big.tile([_P, FREE], f16)
            for s in range(ns):
                a, b = int(off[s]), int(off[s + 1])
                nc.sync.dma_start(out=v_t[:, a:b], in_=v_d[:, a:b])

            junk = big.tile([_P, Lmax], f16)      # DVE fused-op dump
            scr = big.tile([_P, Lmax], f16)       # ACT dump (separate:
            sums = small.tile([_P, ns], f32)      # no cross-engine WAW)
            means = small.tile([_P, ns], f32)
            negm = small.tile([_P, ns], f32)
            devs = small.tile([_P, ns], f32)

            for s in range(ns):
                a, b = int(off[s]), int(off[s + 1])
                if sum_mode == "ts":
                    # accum variant: op1 is the reduce op (add)
                    nc.vector.tensor_scalar(
                        out=junk[:, : Ls[s]], in0=v_t[:, a:b],
                        scalar1=1.0, scalar2=None, op0=Alu.mult,
                        op1=Alu.add, accum_out=sums[:, s:s + 1],
                    )
                else:
                    nc.vector.tensor_reduce(
                        out=sums[:, s:s + 1], in_=v_t[:, a:b],
                        axis=Ax.X, op=Alu.add,
                    )
                nc.vector.tensor_tensor(
                    out=means[:, s:s + 1], in0=sums[:, s:s + 1],
                    in1=w_v[:, s:s + 1], op=Alu.mult,
                )
                nc.vector.tensor_tensor(
                    out=negm[:, s:s + 1], in0=sums[:, s:s + 1],
                    in1=negw_v[:, s:s + 1], op=Alu.mult,
                )
                if dev_eng[s] == "dve":
                    nc.vector.tensor_scalar(
                        out=junk[:, : Ls[s]], in0=v_t[:, a:b],
                        scalar1=means[:, s:s + 1], scalar2=None,
                        op0=Alu.subtract,
                    )
                    nc.vector.tensor_reduce(
                        out=devs[:, s:s + 1], in_=junk[:, : Ls[s]],
                        axis=Ax.X, op=Alu.add, apply_absolute_value=True,
                    )
                else:
                    nc.scalar.activation(
                        out=scr[:, : Ls[s]], in_=v_t[:, a:b], func=Act.Abs,
                        bias=negm[:, s:s + 1], scale=1.0,
                        accum_out=devs[:, s:s + 1],
                    )

            # pads contributed |0 - mean| each: subtract npad * |mean|
            absm = small.tile([_P, ns], f32)
            nc.vector.tensor_tensor(
                out=absm[:], in0=means[:], in1=negm[:], op=Alu.max
            )
            nc.vector.tensor_tensor(
                out=absm[:], in0=absm[:], in1=npad_v, op=Alu.mult
            )
            nc.vector.tensor_tensor(
                out=devs[:], in0=devs[:], in1=absm[:], op=Alu.subtract
            )
            nc.vector.tensor_tensor(
                out=devs[:], in0=devs[:], in1=w_v, op=Alu.mult
            )
            part = small.tile([_P, 1], f32)
            nc.vector.tensor_reduce(
                out=part[:], in_=devs[:], axis=Ax.X, op=Alu.add
            )
            nc.sync.dma_start(out=out_d[:, :], in_=part[:])
    nc.finalize()
    return nc


_CACHE = {}


def _get_nc(key):
    if key not in _CACHE:
        _CACHE[key] = _build_nc(key)
    return _CACHE[key]


def _pack(input, rows, cols, seg_ids, num_paths):
    """Host-side sharding: one image per core; segments sorted by length
    and packed into a [ncore, 128, sum(L_s)] fp16 slot grid."""
    B, C, H, W = input.shape
    ppi = num_paths // B
    npix = rows.shape[0]

    bnd = np.searchsorted(seg_ids, np.arange(num_paths + 1)).astype(np.int64)
    seg_lens = np.diff(bnd)                       # [num_paths]
    ns = (ppi + _P - 1) // _P
    lens_c = seg_lens.reshape(B, ppi)
    order = np.argsort(-lens_c, axis=1, kind="stable")   # [B, ppi] desc
    rank = np.empty_like(order)
    np.put_along_axis(rank, order, np.arange(ppi)[None, :].repeat(B, 0), 1)

    sorted_lens = np.take_along_axis(lens_c, order, axis=1)
    Ls = []
    for k in range(ns):
        m = int(sorted_lens[:, k * _P].max()) if k * _P < ppi else 1
        Ls.append(max(32, -(-m // 32) * 32))
    off = np.concatenate([[0], np.cumsum(Ls)]).astype(np.int64)
    FREE = int(off[-1])

    s_all = np.arange(num_paths)
    core = s_all // ppi
    r = rank[core, s_all % ppi]                  # sorted rank within image
    slot = r // _P
    part = r % _P
    base = (core * _P + part) * np.int64(FREE) + off[slot]
    dest = np.repeat(base, seg_lens) + (
        np.arange(npix, dtype=np.int64) - np.repeat(bnd[:-1], seg_lens)
    )
    core_of = np.repeat(core, seg_lens)
    v_p = np.zeros(B * _P * FREE, np.float16)
    v_p[dest] = input[core_of, 0, rows, cols].astype(np.float16)

    # per-(core, partition, slot) counts -> w, -w, npad  (f32 aux)
    cnt = np.zeros((B, _P, ns), np.float32)
    rr = np.arange(ppi)
    for c in range(B):
        cnt[c, rr % _P, rr // _P] = sorted_lens[c]
    w = 1.0 / np.maximum(cnt, 1.0)
    npad = np.asarray(Ls, np.float32)[None, None, :] - cnt
    aux = np.concatenate([w, -w, npad], axis=2).astype(np.float32)
    return v_p.reshape(B, _P, FREE), aux, tuple(Ls)


def kernel(input, rows, cols, seg_ids, _trace=False, _num_paths=_NUM_PATHS,
           _dev_eng=_DEV_ENG, _sum_mode=_SUM_MODE):
    from concourse.bass_utils import run_bass_kernel_spmd

    input = np.ascontiguousarray(np.asarray(input, np.float32))
    rows = np.ascontiguousarray(np.asarray(rows, np.int32))
    cols = np.ascontiguousarray(np.asarray(cols, np.int32))
    seg_ids = np.ascontiguousarray(np.asarray(seg_ids, np.int32))
    B, C, H, W = input.shape

    v_p, aux, Ls = _pack(input, rows, cols, seg_ids, _num_paths)
    nc = _get_nc((Ls, tuple(_dev_eng[: len(Ls)]), _sum_mode))
    in_maps = [{"vP": v_p[i], "auxP": aux[i]} for i in range(B)]
    res = run_bass_kernel_spmd(nc, in_maps, core_ids=list(range(B)), trace=_trace)
    total = sum(float(r["out"].sum()) for r in res.results)
    out = np.float32(total / B)
    if _trace:
        return out, res
    return out


# revision 10
# speedup vs baseline: 1.6837x; 1.1135x over previous
"""CIGLoss (segment_reduce) Trainium2 kernel.

Strategy (data-parallel over batch, per the sharding hint):
  - Each of the 8 NeuronCores owns one image and that image's pixel list
    (segments are image-local: seg // 500 == image).
  - Host-side sharding sorts each image's ~500 segments by length and
    packs them into a [128 partitions, sum(L_s)] fp16 grid: slot s holds
    segments ranked [s*128, (s+1)*128), padded to that slot's own width
    L_s.  Pad entries are 0 and their known contribution |0 - mean| is
    subtracted exactly on device.
  - The value lookup input[b,0,row,col] happens during host packing (this
    toolchain's walrus mis-lowers per-element indirect DMA: one descriptor
    per contiguous dest run, only the run-start offset honored — verified
    by hardware probes).  All heavy reductions run on device:
        sums_s = accum-reduce(v_s)              (DVE fused tensor_scalar)
        mean_s = sums_s * (1/count)             (counts are host metadata)
        dev_s  = accum-reduce(|v_s - mean_s|)   (DVE fused or ACT Abs+accum)
        contrib = (dev_s - npad_s*|mean_s|) / count
    and a final partition reduce to [128,1] per core.
  - Host sums the 8 per-core partials and divides by B.
"""

import numpy as np

_NUM_PATHS = 4000
_P = 128  # SBUF partitions

# dev-pass engine per slot: 'act' (activation Abs + accum) or 'dve'
# (tensor_tensor subtract + tensor_scalar abs accum, two passes).
_DEV_ENG = ("act", "act", "act", "act")
_SUM_MODE = "ts"  # 'ts' (fused tensor_scalar accum) or 'reduce'


def _build_nc(key):
    import concourse.bacc as bacc
    import concourse.bass as bass
    import concourse.tile as tile
    from concourse import mybir

    Ls, dev_eng, sum_mode = key
    f32 = mybir.dt.float32
    f16 = mybir.dt.float16
    Alu = mybir.AluOpType
    Ax = mybir.AxisListType
    Act = mybir.ActivationFunctionType
    ns = len(Ls)
    off = np.concatenate([[0], np.cumsum(Ls)]).astype(int)
    FREE = int(off[-1])
    Lmax = max(Ls)

    nc = bacc.Bacc("TRN2", debug=False)
    v_d = nc.dram_tensor("vP", [_P, FREE], f16, kind="ExternalInput")
    aux_d = nc.dram_tensor("auxP", [_P, 3 * ns], f32, kind="ExternalInput")
    out_d = nc.dram_tensor("out", [1, ns], f32, kind="ExternalOutput")

    any_act = any(e == "act" for e in dev_eng)

    with tile.TileContext(nc) as tc:
        with (
            tc.tile_pool(name="big", bufs=1) as big,
            tc.tile_pool(name="small", bufs=1) as small,
            tc.tile_pool(name="psum", bufs=1, space="PSUM") as psum,
        ):
            # input DMAs first so slot 0 lands ASAP
            v_t = big.tile([_P, FREE], f16)
            for s in range(ns):
                a, b = int(off[s]), int(off[s + 1])
                nc.sync.dma_start(out=v_t[:, a:b], in_=v_d[:, a:b])
            aux_t = small.tile([_P, 3 * ns], f32)
            nc.sync.dma_start(out=aux_t[:], in_=aux_d[:, :])
            w_v = aux_t[:, 0:ns]
            negw_v = aux_t[:, ns:2 * ns]
            npad_v = aux_t[:, 2 * ns:3 * ns]

            ones_t = small.tile([_P, 1], f32)
            nc.vector.memset(ones_t[:], 1.0)
            if any_act:
                # trigger the ACT table-set load early so it overlaps the
                # DMA lead-in instead of stalling the first real dev pass
                warm = small.tile([_P, 1], f32)
                warmacc = small.tile([_P, 1], f32)
                nc.vector.memset(warm[:], 0.0)
                nc.scalar.activation(
                    out=warm[:], in_=warm[:], func=Act.Abs,
                    bias=0.0, scale=1.0, accum_out=warmacc[:],
                )

            junk = big.tile([_P, Lmax], f16)      # DVE fused-op dump
            scr = big.tile([_P, Lmax], f16)       # ACT dump (separate:
            sums = small.tile([_P, ns], f32)      # no cross-engine WAW)
            means = small.tile([_P, ns], f32)
            negm = small.tile([_P, ns], f32)
            devs = small.tile([_P, ns], f32)

            for s in range(ns):
                a, b = int(off[s]), int(off[s + 1])
                if sum_mode == "ts":
                    # accum variant: op1 is the reduce op (add)
                    nc.vector.tensor_scalar(
                        out=junk[:, : Ls[s]], in0=v_t[:, a:b],
                        scalar1=1.0, scalar2=None, op0=Alu.mult,
                        op1=Alu.add, accum_out=sums[:, s:s + 1],
                    )
                else:
                    nc.vector.tensor_reduce(
                        out=sums[:, s:s + 1], in_=v_t[:, a:b],
                        axis=Ax.X, op=Alu.add,
                    )
                nc.vector.tensor_tensor(
                    out=means[:, s:s + 1], in0=sums[:, s:s + 1],
                    in1=w_v[:, s:s + 1], op=Alu.mult,
                )
                nc.vector.tensor_tensor(
                    out=negm[:, s:s + 1], in0=sums[:, s:s + 1],
                    in1=negw_v[:, s:s + 1], op=Alu.mult,
                )
                if dev_eng[s] == "dve":
                    nc.vector.tensor_scalar(
                        out=junk[:, : Ls[s]], in0=v_t[:, a:b],
                        scalar1=means[:, s:s + 1], scalar2=None,
                        op0=Alu.subtract,
                    )
                    nc.vector.tensor_reduce(
                        out=devs[:, s:s + 1], in_=junk[:, : Ls[s]],
                        axis=Ax.X, op=Alu.add, apply_absolute_value=True,
                    )
                else:
                    nc.scalar.activation(
                        out=scr[:, : Ls[s]], in_=v_t[:, a:b], func=Act.Abs,
                        bias=negm[:, s:s + 1], scale=1.0,
                        accum_out=devs[:, s:s + 1],
                    )

            # pads contributed |0 - mean| each: subtract npad * |mean|
            absm = small.tile([_P, ns], f32)
            nc.vector.tensor_tensor(
                out=absm[:], in0=means[:], in1=negm[:], op=Alu.max
            )
            nc.vector.tensor_tensor(
                out=absm[:], in0=absm[:], in1=npad_v, op=Alu.mult
            )
            nc.vector.tensor_tensor(
                out=devs[:], in0=devs[:], in1=absm[:], op=Alu.subtract
            )
            nc.vector.tensor_tensor(
                out=devs[:], in0=devs[:], in1=w_v, op=Alu.mult
            )
            # partition-reduce on the PE so the output is a [1, ns] row:
            # a [128,1] store would shatter into 16 per-SDMA-engine
            # slivers whose completion incs trickle in over ~5us
            part_p = psum.tile([1, ns], f32)
            nc.tensor.matmul(
                part_p[:], ones_t[:], devs[:], start=True, stop=True
            )
            part = small.tile([1, ns], f32)
            nc.vector.tensor_copy(out=part[:], in_=part_p[:])
            nc.sync.dma_start(out=out_d[:, :], in_=part[:])
    nc.finalize()
    return nc


_CACHE = {}


def _get_nc(key):
    if key not in _CACHE:
        _CACHE[key] = _build_nc(key)
    return _CACHE[key]


def _pack(input, rows, cols, seg_ids, num_paths):
    """Host-side sharding: one image per core; segments sorted by length
    and packed into a [ncore, 128, sum(L_s)] fp16 slot grid."""
    B, C, H, W = input.shape
    ppi = num_paths // B
    npix = rows.shape[0]

    bnd = np.searchsorted(seg_ids, np.arange(num_paths + 1)).astype(np.int64)
    seg_lens = np.diff(bnd)                       # [num_paths]
    ns = (ppi + _P - 1) // _P
    lens_c = seg_lens.reshape(B, ppi)
    order = np.argsort(-lens_c, axis=1, kind="stable")   # [B, ppi] desc
    rank = np.empty_like(order)
    np.put_along_axis(rank, order, np.arange(ppi)[None, :].repeat(B, 0), 1)

    sorted_lens = np.take_along_axis(lens_c, order, axis=1)
    Ls = []
    for k in range(ns):
        m = int(sorted_lens[:, k * _P].max()) if k * _P < ppi else 1
        Ls.append(max(32, -(-m // 32) * 32))
    off = np.concatenate([[0], np.cumsum(Ls)]).astype(np.int64)
    FREE = int(off[-1])

    s_all = np.arange(num_paths)
    core = s_all // ppi
    r = rank[core, s_all % ppi]                  # sorted rank within image
    slot = r // _P
    part = r % _P
    base = (core * _P + part) * np.int64(FREE) + off[slot]
    dest = np.repeat(base, seg_lens) + (
        np.arange(npix, dtype=np.int64) - np.repeat(bnd[:-1], seg_lens)
    )
    core_of = np.repeat(core, seg_lens)
    v_p = np.zeros(B * _P * FREE, np.float16)
    v_p[dest] = input[core_of, 0, rows, cols].astype(np.float16)

    # per-(core, partition, slot) counts -> w, -w, npad  (f32 aux)
    cnt = np.zeros((B, _P, ns), np.float32)
    rr = np.arange(ppi)
    for c in range(B):
        cnt[c, rr % _P, rr // _P] = sorted_lens[c]
    w = 1.0 / np.maximum(cnt, 1.0)
    npad = np.asarray(Ls, np.float32)[None, None, :] - cnt
    aux = np.concatenate([w, -w, npad], axis=2).astype(np.float32)
    return v_p.reshape(B, _P, FREE), aux, tuple(Ls)


def kernel(input, rows, cols, seg_ids, _trace=False, _num_paths=_NUM_PATHS,
           _dev_eng=_DEV_ENG, _sum_mode=_SUM_MODE):
    from concourse.bass_utils import run_bass_kernel_spmd

    input = np.ascontiguousarray(np.asarray(input, np.float32))
    rows = np.ascontiguousarray(np.asarray(rows, np.int32))
    cols = np.ascontiguousarray(np.asarray(cols, np.int32))
    seg_ids = np.ascontiguousarray(np.asarray(seg_ids, np.int32))
    B, C, H, W = input.shape

    v_p, aux, Ls = _pack(input, rows, cols, seg_ids, _num_paths)
    nc = _get_nc((Ls, tuple(_dev_eng[: len(Ls)]), _sum_mode))
    in_maps = [{"vP": v_p[i], "auxP": aux[i]} for i in range(B)]
    res = run_bass_kernel_spmd(nc, in_maps, core_ids=list(range(B)), trace=_trace)
    total = sum(float(r["out"].sum()) for r in res.results)
    out = np.float32(total / B)
    if _trace:
        return out, res
    return out


# revision 11
# speedup vs baseline: 1.9779x; 1.1747x over previous
"""CIGLoss (segment_reduce) Trainium2 kernel.

Strategy (data-parallel over batch, per the sharding hint):
  - Each of the 8 NeuronCores owns one image and that image's pixel list
    (segments are image-local: seg // 500 == image).
  - Host-side sharding sorts each image's ~500 segments by length
    (ascending) and packs them into a [128 partitions, sum(L_s)] fp16
    grid: slot s holds segments ranked [s*128, (s+1)*128), padded to
    that slot's own width L_s.  Pad entries are 0 and their known
    contribution |0 - mean| is subtracted exactly on device.
  - The value lookup input[b,0,row,col] happens during host packing (this
    toolchain's walrus mis-lowers per-element indirect DMA: one descriptor
    per contiguous dest run, only the run-start offset honored — verified
    by hardware probes).  All heavy reductions run on device:
        sums_s = accum-reduce(v_s)       slot 0 on ACT, rest on DVE
        mean_s = sums_s * (1/count)      (counts are host metadata)
        dev_s  = ACT Abs activation with bias=-mean_s, accum_out
        contrib_s = (dev_s - npad_s*|mean_s|) / count
    then a PE ones-matmul partition-reduce so the output is a single
    [1, nslot] row (a [128,1] store shatters into 16 per-SDMA-engine
    slivers whose completion semaphores trickle in over ~5us).
  - Host sums the 8 per-core partials and divides by B.

Schedule notes (from perfetto traces):
  - aux is DMA'd from the Scalar queue in parallel with the v DMAs on
    Sync; if aux lands late the scheduler hoists all DVE sums ahead of
    the aux-dependent mean ops, serializing the ACT chain behind them.
  - A dummy ACT op up front pulls the ~1.3us ACT_TABLE_LOAD into the
    DMA lead-in.
  - Slot 0 is the smallest slot and its sum runs on ACT so the ACT
    Abs chain (the critical path) starts as early as possible.
"""

import numpy as np

_NUM_PATHS = 4000
_P = 128  # SBUF partitions


def _build_nc(key):
    import concourse.bacc as bacc
    import concourse.bass as bass
    import concourse.tile as tile
    from concourse import mybir

    (Ls,) = key
    f32 = mybir.dt.float32
    f16 = mybir.dt.float16
    Alu = mybir.AluOpType
    Ax = mybir.AxisListType
    Act = mybir.ActivationFunctionType
    ns = len(Ls)
    off = np.concatenate([[0], np.cumsum(Ls)]).astype(int)
    FREE = int(off[-1])
    Lmax = max(Ls)

    nc = bacc.Bacc("TRN2", debug=False)
    v_d = nc.dram_tensor("vP", [_P, FREE], f16, kind="ExternalInput")
    aux_d = nc.dram_tensor("auxP", [_P, 3 * ns], f32, kind="ExternalInput")
    out_d = nc.dram_tensor("out", [1, ns], f32, kind="ExternalOutput")

    with tile.TileContext(nc) as tc:
        with (
            tc.tile_pool(name="big", bufs=1) as big,
            tc.tile_pool(name="small", bufs=1) as small,
            tc.tile_pool(name="psum", bufs=1, space="PSUM") as psum,
        ):
            # input DMAs first: v slots on Sync, aux on Scalar (parallel)
            v_t = big.tile([_P, FREE], f16)
            for s in range(ns):
                a, b = int(off[s]), int(off[s + 1])
                nc.sync.dma_start(out=v_t[:, a:b], in_=v_d[:, a:b])
            aux_t = small.tile([_P, 3 * ns], f32)
            nc.scalar.dma_start(out=aux_t[:], in_=aux_d[:, :])
            w_v = aux_t[:, 0:ns]
            negw_v = aux_t[:, ns:2 * ns]
            npad_v = aux_t[:, 2 * ns:3 * ns]

            ones_t = small.tile([_P, 1], f32)
            nc.vector.memset(ones_t[:], 1.0)
            # trigger the ACT table-set load during the DMA lead-in
            warm = small.tile([_P, 1], f32)
            warmacc = small.tile([_P, 1], f32)
            nc.vector.memset(warm[:], 0.0)
            nc.scalar.activation(
                out=warm[:], in_=warm[:], func=Act.Abs,
                bias=0.0, scale=1.0, accum_out=warmacc[:],
            )

            junk = big.tile([_P, Lmax], f16)      # DVE fused-op dump
            scr = big.tile([_P, Lmax], f16)       # ACT dump (separate:
            sums = small.tile([_P, ns], f32)      # no cross-engine WAW)
            means = small.tile([_P, ns], f32)
            negm = small.tile([_P, ns], f32)
            devs = small.tile([_P, ns], f32)
            contrib = small.tile([_P, ns], f32)

            def dve_sum(s):
                a, b = int(off[s]), int(off[s + 1])
                nc.vector.tensor_scalar(
                    out=junk[:, : Ls[s]], in0=v_t[:, a:b],
                    scalar1=1.0, scalar2=None, op0=Alu.mult,
                    op1=Alu.add, accum_out=sums[:, s:s + 1],
                )

            def dve_mn(s):
                nc.vector.tensor_tensor(
                    out=means[:, s:s + 1], in0=sums[:, s:s + 1],
                    in1=w_v[:, s:s + 1], op=Alu.mult,
                )
                nc.vector.tensor_tensor(
                    out=negm[:, s:s + 1], in0=sums[:, s:s + 1],
                    in1=negw_v[:, s:s + 1], op=Alu.mult,
                )

            def act_dev(s):
                a, b = int(off[s]), int(off[s + 1])
                nc.scalar.activation(
                    out=scr[:, : Ls[s]], in_=v_t[:, a:b], func=Act.Abs,
                    bias=negm[:, s:s + 1], scale=1.0,
                    accum_out=devs[:, s:s + 1],
                )

            def dve_tail(s):
                # contrib_s = (devs_s - npad_s*max(means_s,-means_s)) * w_s
                am = contrib[:, s:s + 1]
                nc.vector.tensor_tensor(
                    out=am, in0=means[:, s:s + 1], in1=negm[:, s:s + 1],
                    op=Alu.max,
                )
                nc.vector.tensor_tensor(
                    out=am, in0=am, in1=npad_v[:, s:s + 1], op=Alu.mult
                )
                nc.vector.tensor_tensor(
                    out=am, in0=devs[:, s:s + 1], in1=am, op=Alu.subtract
                )
                nc.vector.tensor_tensor(
                    out=am, in0=am, in1=w_v[:, s:s + 1], op=Alu.mult
                )

            # slot 0: sum + negmean on ACT so the Abs chain starts early
            # (no cross-engine hop before ABS0)
            nc.scalar.activation(
                out=scr[:, : Ls[0]], in_=v_t[:, int(off[0]):int(off[1])],
                func=Act.Copy, bias=0.0, scale=1.0,
                accum_out=sums[:, 0:1],
            )
            nc.scalar.activation(
                out=negm[:, 0:1], in_=sums[:, 0:1], func=Act.Copy,
                bias=0.0, scale=negw_v[:, 0:1],
            )
            act_dev(0)
            nc.vector.tensor_tensor(
                out=means[:, 0:1], in0=sums[:, 0:1], in1=w_v[:, 0:1],
                op=Alu.mult,
            )
            for s in range(1, ns):
                dve_sum(s)
                dve_mn(s)
                act_dev(s)
                dve_tail(s - 1)
            dve_tail(ns - 1)

            # partition-reduce on the PE -> [1, ns] single-descriptor out
            part_p = psum.tile([1, ns], f32)
            nc.tensor.matmul(
                part_p[:], ones_t[:], contrib[:], start=True, stop=True
            )
            part = small.tile([1, ns], f32)
            nc.vector.tensor_copy(out=part[:], in_=part_p[:])
            nc.sync.dma_start(out=out_d[:, :], in_=part[:])
    nc.finalize()
    return nc


_CACHE = {}


def _get_nc(key):
    if key not in _CACHE:
        _CACHE[key] = _build_nc(key)
    return _CACHE[key]


def _pack(input, rows, cols, seg_ids, num_paths):
    """Host-side sharding: one image per core; segments sorted by length
    (ascending) and packed into a [ncore, 128, sum(L_s)] fp16 slot grid."""
    B, C, H, W = input.shape
    ppi = num_paths // B
    npix = rows.shape[0]

    bnd = np.searchsorted(seg_ids, np.arange(num_paths + 1)).astype(np.int64)
    seg_lens = np.diff(bnd)                       # [num_paths]
    ns = (ppi + _P - 1) // _P
    lens_c = seg_lens.reshape(B, ppi)
    order = np.argsort(lens_c, axis=1, kind="stable")    # [B, ppi] asc
    rank = np.empty_like(order)
    np.put_along_axis(rank, order, np.arange(ppi)[None, :].repeat(B, 0), 1)

    sorted_lens = np.take_along_axis(lens_c, order, axis=1)
    Ls = []
    for k in range(ns):
        blk = sorted_lens[:, k * _P:(k + 1) * _P]
        m = int(blk.max()) if blk.size else 1
        Ls.append(max(32, -(-m // 32) * 32))
    off = np.concatenate([[0], np.cumsum(Ls)]).astype(np.int64)
    FREE = int(off[-1])

    s_all = np.arange(num_paths)
    core = s_all // ppi
    r = rank[core, s_all % ppi]                  # sorted rank within image
    slot = r // _P
    part = r % _P
    base = (core * _P + part) * np.int64(FREE) + off[slot]
    dest = np.repeat(base, seg_lens) + (
        np.arange(npix, dtype=np.int64) - np.repeat(bnd[:-1], seg_lens)
    )
    core_of = np.repeat(core, seg_lens)
    v_p = np.zeros(B * _P * FREE, np.float16)
    v_p[dest] = input[core_of, 0, rows, cols].astype(np.float16)

    # per-(core, partition, slot) counts -> w, -w, npad  (f32 aux)
    cnt = np.zeros((B, _P, ns), np.float32)
    rr = np.arange(ppi)
    for c in range(B):
        cnt[c, rr % _P, rr // _P] = sorted_lens[c]
    w = 1.0 / np.maximum(cnt, 1.0)
    npad = np.asarray(Ls, np.float32)[None, None, :] - cnt
    aux = np.concatenate([w, -w, npad], axis=2).astype(np.float32)
    return v_p.reshape(B, _P, FREE), aux, tuple(Ls)


def kernel(input, rows, cols, seg_ids, _trace=False, _num_paths=_NUM_PATHS):
    from concourse.bass_utils import run_bass_kernel_spmd

    input = np.ascontiguousarray(np.asarray(input, np.float32))
    rows = np.ascontiguousarray(np.asarray(rows, np.int32))
    cols = np.ascontiguousarray(np.asarray(cols, np.int32))
    seg_ids = np.ascontiguousarray(np.asarray(seg_ids, np.int32))
    B, C, H, W = input.shape

    v_p, aux, Ls = _pack(input, rows, cols, seg_ids, _num_paths)
    nc = _get_nc((Ls,))
    in_maps = [{"vP": v_p[i], "auxP": aux[i]} for i in range(B)]
    res = run_bass_kernel_spmd(nc, in_maps, core_ids=list(range(B)), trace=_trace)
    total = sum(float(r["out"].sum()) for r in res.results)
    out = np.float32(total / B)
    if _trace:
        return out, res
    return out
